# revision 10
# baseline (speedup 1.0000x reference)
"""EnsembleRBF Trainium2 kernel: out[m,n,d] = sum_c exp(-||x_n - c_c||^2) * sigma^2 * w[m,c,d].

Rank-reduced design:
  The output is 10 fixed smooth functions f_{m,d}(x) = sigma^2 * sum_c
  exp(-||x-c_c||^2) w[m,c,d]. On host, select R=64 of the 256 kernel columns
  by greedy pivoted-QR over a dense x-grid (interpolative decomposition) and
  least-squares fit coefficients G[64,10] so sum_r exp(-||x-a_r||^2) G[r,:]
  matches all 10 targets to ~2e-4 (gate is 2e-2). Device exp work shrinks
  4x: [n, 64] instead of [n, 256].

  Data-parallel along N across 8 cores, NCP = 12544 = 128 x 98 blocks,
  n = p*98 + b. Two point-sets packed on the partition axis keep ACT full
  width: d2 tile [128, 1536] fp32 PSUM (3 banks), partitions 0:64 = 64
  anchors x even blocks (set A), 64:128 = anchors x odd blocks (set B);
  col-block cb of tile t holds blocks 24t+2cb (A) / 24t+2cb+1 (B).

  MM1 (PE): 6 matmuls per tile (FD=512, K=10 fp16 hi/lo features); three rx
  chains at partition bases 0/32/64 rotate across row-quadrants so the three
  A matmuls run concurrently (per-subarray row tiling), B likewise.

  ACT: one exp(-d2) per tile (FD=1536) = the stream bottleneck; 4 full tiles
  + 1 tail (2 blocks, FD=128). The scalar queue carries ONLY the dummy exp
  (hoists the ~2.7us table load to t~0) and the exps - DMA triggers cost
  ~740ns of sequencer time each and must live on other queues.

  MM2 (PE): ONE matmul per col-block computes BOTH packed blocks: stationary
  rbf[:, 128cb:+128] (K=128), moving wg[:, 0:32] with cols 0:16 = [G; 0]
  (kills the B rows) and 16:32 = [0; G] (kills A) -> po[j, 32cb+16a+2m+d].
  49 matmuls total instead of 98; per-PE-instruction cost (~120ns dispatch+
  LDW+MM) dominates, so count is everything. (K=64 stationaries at partition
  base 64 crash the HW - all-row zero-padded contraction is the workaround.)

  DVE: po -> stage (m,b,d)-major fp32. 5 output waves on the gpsimd queue,
  each one 4D-AP DMA covering all 5 models ([p][m][b][d] element order).
"""
import numpy as np

import concourse.bass as bass
import concourse.tile as tile
from concourse import bacc, mybir
from concourse.bass_utils import run_bass_kernel_spmd

N, C, D, M = 100000, 256, 2, 5
SIGMA2 = 0.0625
NCORES = 8
NCP = 12544          # padded rows per core (128 x 98)
NBLK = NCP // 128    # 98 blocks, n = p*98 + b
R = 64               # anchor count
f32 = mybir.dt.float32
f16 = mybir.dt.float16

NT = 4               # full tiles (24 blocks each); tail = blocks 96,97
CH0_COLS = 4 * 1024 + 256   # chain0 also carries the tail blocks
CH_COLS = 4 * 1024

_CACHE = {}


def _build():
    nc = bacc.Bacc("TRN2", target_bir_lowering=False, debug=False, num_devices=1)
    rx_ap = nc.dram_tensor("rx", [74, CH0_COLS], f16, kind="ExternalInput").ap()
    cw_ap = nc.dram_tensor("cw", [128, 160], f16, kind="ExternalInput").ap()
    out_ap = nc.dram_tensor("out", [M, NCP, 2], f32, kind="ExternalOutput").ap()

    Exp = mybir.ActivationFunctionType.Exp

    with tile.TileContext(nc) as tc:
        with (
            tc.tile_pool(name="consts", bufs=1) as consts,
            tc.tile_pool(name="d2p", bufs=2, space="PSUM") as d2_pool,
            tc.tile_pool(name="pop", bufs=2, space="PSUM") as po_pool,
        ):
            cw = consts.tile([128, 160], f16)
            rxsb = consts.tile([128, CH0_COLS], f16)
            rbf = consts.tile([128, 4 * 1536 + 256], f16)
            stage = consts.tile([128, M * NBLK * 2], f32)
            dum_i = consts.tile([128, 1], f32)
            dum_o = consts.tile([128, 1], f16)

            augw = cw[:, 0:128]
            wg = cw[:, 128:160]

            # memset first (dummy exp depends on it), then DMA triggers on
            # the vector queue; scalar queue = dummy + exps ONLY
            nc.vector.memset(dum_i[:], 0.0)
            nc.scalar.activation(dum_o[:], dum_i[:], Exp, scale=-1.0)
            # per-chain piece DMAs: a single batched transfer engages only ~2
            # DMA engines and serializes tiles 1-3 behind one completion sem;
            # per-piece DMAs spread across 10+ engines (scalar stays exp-only)
            nc.sync.dma_start(cw[:], cw_ap[:])
            nc.sync.dma_start(rxsb[0:10, 0:1024], rx_ap[0:10, 0:1024])
            nc.sync.dma_start(rxsb[64:74, 0:1024], rx_ap[64:74, 0:1024])
            nc.gpsimd.dma_start(rxsb[32:42, 0:1024], rx_ap[32:42, 0:1024])
            nc.gpsimd.dma_start(rxsb[32:42, 1024:4096], rx_ap[32:42, 1024:4096])
            nc.sync.dma_start(rxsb[0:10, 1024:CH0_COLS], rx_ap[0:10, 1024:CH0_COLS])
            nc.sync.dma_start(rxsb[64:74, 1024:4096], rx_ap[64:74, 1024:4096])

            stv = stage[:].rearrange("p (m b d) -> p m b d", m=M, d=2)

            def mm1(t):
                # 3 chains x (A then B); A matmuls on distinct row-quadrants
                # run concurrently, B likewise (T2-verified: same-bank writes
                # from different quadrants are safe)
                d2 = d2_pool.tile([128, 1536], f32, tag="d2")
                for k, r0 in enumerate((0, 32, 64)):
                    nc.tensor.matmul(
                        d2[0:64, 512 * k : 512 * (k + 1)],
                        augw[r0 : r0 + 10, 0:64],
                        rxsb[r0 : r0 + 10, 1024 * t : 1024 * t + 512],
                        start=True,
                        stop=True,
                    )
                    nc.tensor.matmul(
                        d2[64:128, 512 * k : 512 * (k + 1)],
                        augw[r0 : r0 + 10, 64:128],
                        rxsb[r0 : r0 + 10, 1024 * t + 512 : 1024 * (t + 1)],
                        start=True,
                        stop=True,
                    )
                return d2

            def mm1_tail():
                d2 = d2_pool.tile([128, 1536], f32, tag="d2")
                nc.tensor.matmul(
                    d2[0:64, 0:128],
                    augw[0:10, 0:64],
                    rxsb[0:10, 4096:4224],
                    start=True,
                    stop=True,
                )
                nc.tensor.matmul(
                    d2[64:128, 0:128],
                    augw[0:10, 64:128],
                    rxsb[0:10, 4224:4352],
                    start=True,
                    stop=True,
                )
                return d2

            def mm2(t):
                # one matmul per col-block computes both packed blocks
                po = po_pool.tile([128, 32 * 12], f32, tag="po")
                for cb in range(12):
                    nc.tensor.matmul(
                        po[:, 32 * cb : 32 * cb + 32],
                        rbf[:, 1536 * t + 128 * cb : 1536 * t + 128 * cb + 128],
                        wg[:, 0:32],
                        start=True,
                        stop=True,
                    )
                pov = po[:].rearrange("p (i a m d) -> p m (i a) d", a=2, m=8, d=2)
                nc.vector.tensor_copy(
                    stv[:, :, 24 * t : 24 * t + 24, :], pov[:, 0:M, 0:24, :]
                )

            def mm2_tail():
                po = po_pool.tile([128, 32 * 12], f32, tag="po")
                nc.tensor.matmul(
                    po[:, 0:32], rbf[:, 6144:6272], wg[:, 0:32],
                    start=True, stop=True,
                )
                pov = po[:].rearrange("p (i a m d) -> p m (i a) d", a=2, m=8, d=2)
                nc.vector.tensor_copy(stv[:, :, 96:98, :], pov[:, 0:M, 0:2, :])

            def wave(blo, bhi):
                dst = out_ap.rearrange("m (p b) d -> p m b d", p=128)[
                    :, :, blo:bhi, :
                ]
                nc.sync.dma_start(dst, stv[:, :, blo:bhi, :])

            def do_exp(t, d2):
                fd = 1536 if t < NT else 128
                off = 1536 * t
                nc.scalar.activation(
                    rbf[:, off : off + fd], d2[:, 0:fd], Exp, scale=-1.0
                )

            d2 = mm1(0)
            do_exp(0, d2)
            d2 = mm1(1)
            do_exp(1, d2)
            for t in range(2, NT + 1):
                d2 = mm1(t) if t < NT else mm1_tail()
                do_exp(t, d2)
                mm2(t - 2)
                wave(24 * (t - 2), 24 * (t - 1))
            mm2(NT - 1)
            mm2_tail()
            wave(24 * (NT - 1), NBLK)

    nc.compile()
    return nc


def _fit_anchors(centers, weights, xmax):
    """Interpolative decomposition of K(x, c) over a dense grid + LS fit of
    the 10 target functions on the selected anchor columns."""
    L = max(5.1, xmax + 0.35)
    ng = 96
    g1 = np.linspace(-L, L, ng)
    G2 = np.stack(np.meshgrid(g1, g1, indexing="ij"), -1).reshape(-1, 2)
    Kg = np.exp(-((G2[:, None, :] - centers[None, :, :]) ** 2).sum(-1))

    res = Kg.copy()
    sel = []
    for _ in range(R):
        j = int(np.argmax((res * res).sum(0)))
        sel.append(j)
        q = res[:, j].copy()
        nq = float(np.linalg.norm(q))
        if nq < 1e-12:
            break
        q /= nq
        res -= np.outer(q, q @ res)
    while len(sel) < R:          # degenerate guard: pad with repeats
        sel.append(sel[-1])

    V = weights.transpose(1, 0, 2).reshape(C, 10).astype(np.float64)
    F = SIGMA2 * (Kg @ V)
    A = Kg[:, sel]
    GA = A.T @ A + 1e-12 * np.eye(R)
    Gc = np.linalg.solve(GA, A.T @ F)          # [R, 10]
    return centers[sel].astype(np.float32), Gc.astype(np.float32)


def _host_prep(x, centers, weights):
    x = np.ascontiguousarray(np.asarray(x, dtype=np.float32))
    centers = np.asarray(centers, dtype=np.float32)
    weights = np.asarray(weights, dtype=np.float32)

    anchors, Gc = _fit_anchors(centers, weights, float(np.abs(x).max()))

    xp = np.zeros((NCORES * NCP, 2), np.float32)
    xp[:N] = x

    # x-side features, fp16 hi/lo split
    xh = xp.astype(np.float16)
    xl = (xp - xh.astype(np.float32)).astype(np.float16)
    x2 = np.sum(xp * xp, axis=1, dtype=np.float32)
    x2h = x2.astype(np.float16)
    x2l = (x2 - x2h.astype(np.float32)).astype(np.float16)
    ones = np.ones(NCORES * NCP, np.float16)
    feats = np.stack([
        xh[:, 0], xh[:, 0], xl[:, 0], xh[:, 1], xh[:, 1], xl[:, 1],
        x2h, x2l, ones, ones,
    ])  # [10, NCORES*NCP]
    fv = feats.reshape(10, NCORES, 128, NBLK)  # [k, core, j(p), b]

    # chain k, tile t: A-blocks {24t+2cb} then B-blocks {24t+2cb+1} for
    # cb in [4k, 4k+4); chain0 tail: blocks 96 (A), 97 (B)
    chains = []
    for k in range(3):
        blks = []
        for t in range(NT):
            blks += [24 * t + 2 * cb for cb in range(4 * k, 4 * k + 4)]
            blks += [24 * t + 2 * cb + 1 for cb in range(4 * k, 4 * k + 4)]
        if k == 0:
            blks += [96, 97]
        rc = fv[:, :, :, blks]                       # [10, core, j, nb]
        rc = np.ascontiguousarray(rc.transpose(1, 0, 3, 2))
        chains.append(rc.reshape(NCORES, 10, -1))    # [core, 10, 128*nb]
    rxp = np.zeros((NCORES, 74, CH0_COLS), np.float16)
    rxp[:, 0:10, :] = chains[0]
    rxp[:, 32:42, 0:CH_COLS] = chains[1]
    rxp[:, 64:74, 0:CH_COLS] = chains[2]

    # anchor-side features (both column halves hold the same 64 anchors)
    ch = anchors.astype(np.float16)
    cl = (anchors - ch.astype(np.float32)).astype(np.float16)
    c2 = np.sum(anchors * anchors, axis=1, dtype=np.float32)
    c2h = c2.astype(np.float16)
    c2l = (c2 - c2h.astype(np.float32)).astype(np.float16)
    onesR = np.ones(R, np.float16)
    cf = np.stack([
        -2 * ch[:, 0], -2 * cl[:, 0], -2 * ch[:, 0],
        -2 * ch[:, 1], -2 * cl[:, 1], -2 * ch[:, 1],
        onesR, onesR, c2h, c2l,
    ])  # [10, 64]
    cwm = np.zeros((128, 160), np.float16)
    for r0 in (0, 32, 64):
        cwm[r0 : r0 + 10, 0:64] = cf
        cwm[r0 : r0 + 10, 64:128] = cf
    # wg cols 0:16 = [G; 0] (A blocks), 16:32 = [0; G] (B blocks): one K=128
    # matmul per col-block computes both packed point-sets; the zero half
    # cancels the other set's rows.
    cwm[0:R, 128:138] = Gc.astype(np.float16)
    cwm[64 : 64 + R, 144:154] = Gc.astype(np.float16)
    return rxp, cwm


def kernel(x, centers, weights):
    if "nc" not in _CACHE:
        _CACHE["nc"] = _build()
    nc = _CACHE["nc"]
    rxp, cwm = _host_prep(x, centers, weights)
    in_maps = [{"rx": rxp[i], "cw": cwm} for i in range(NCORES)]
    res = run_bass_kernel_spmd(nc, in_maps, list(range(NCORES)))
    outs = np.concatenate([res.results[i]["out"] for i in range(NCORES)], axis=1)
    return np.ascontiguousarray(outs[:, :N, :])


# revision 11
# speedup vs baseline: 1.2433x; 1.2433x over previous
"""EnsembleRBF Trainium2 kernel: out[m,n,d] = sum_c exp(-||x_n - c_c||^2) * sigma^2 * w[m,c,d].

Rank-reduced design:
  The output is 10 fixed smooth functions f_{m,d}(x) = sigma^2 * sum_c
  exp(-||x-c_c||^2) w[m,c,d]. On host, select R=64 of the 256 kernel columns
  by greedy pivoted-QR over a dense x-grid (interpolative decomposition) and
  least-squares fit coefficients G[64,10] so sum_r exp(-||x-a_r||^2) G[r,:]
  matches all 10 targets to ~2e-4 (gate is 2e-2). Device exp work shrinks
  4x: [n, 64] instead of [n, 256].

  Data-parallel along N across 8 cores, NCP = 12544 = 128 x 98 blocks,
  n = p*98 + b. Two point-sets packed on the partition axis keep ACT full
  width: d2 tile [128, 1536] fp32 PSUM (3 banks), partitions 0:64 = 64
  anchors x even blocks (set A), 64:128 = anchors x odd blocks (set B);
  col-block cb of tile t holds blocks 24t+2cb (A) / 24t+2cb+1 (B).

  MM1 (PE): 6 matmuls per tile (FD=512, K=10 fp16 hi/lo features); three rx
  chains at partition bases 0/32/64 rotate across row-quadrants so the three
  A matmuls run concurrently (per-subarray row tiling), B likewise.

  ACT: one exp(-d2) per tile (FD=1536) = the stream bottleneck; 4 full tiles
  + 1 tail (2 blocks, FD=128). The scalar queue carries ONLY the dummy exp
  (hoists the ~2.7us table load to t~0) and the exps - DMA triggers cost
  ~740ns of sequencer time each and must live on other queues.

  MM2 (PE): ONE matmul per col-block computes BOTH packed blocks: stationary
  rbf[:, 128cb:+128] (K=128), moving wg[:, 0:32] with cols 0:16 = [G; 0]
  (kills the B rows) and 16:32 = [0; G] (kills A) -> po[j, 32cb+16a+2m+d].
  49 matmuls total instead of 98; per-PE-instruction cost (~120ns dispatch+
  LDW+MM) dominates, so count is everything. (K=64 stationaries at partition
  base 64 crash the HW - all-row zero-padded contraction is the workaround.)

  DVE: po -> stage (m,b,d)-major fp32. 5 output waves on the gpsimd queue,
  each one 4D-AP DMA covering all 5 models ([p][m][b][d] element order).
"""
import numpy as np

import concourse.bass as bass
import concourse.tile as tile
from concourse import bacc, mybir
from concourse.bass_utils import run_bass_kernel_spmd

N, C, D, M = 100000, 256, 2, 5
SIGMA2 = 0.0625
NCORES = 8
NCP = 12544          # padded rows per core (128 x 98)
NBLK = NCP // 128    # 98 blocks, n = p*98 + b
R = 64               # anchor count
f32 = mybir.dt.float32
f16 = mybir.dt.float16

NT = 4               # full tiles (24 blocks each); tail = blocks 96,97
CH0_COLS = 4 * 1024 + 256   # chain0 also carries the tail blocks
CH_COLS = 4 * 1024

_CACHE = {}


def _build():
    nc = bacc.Bacc("TRN2", target_bir_lowering=False, debug=False, num_devices=NCORES)
    rx_ap = nc.dram_tensor("rx", [74, CH0_COLS], f16, kind="ExternalInput").ap()
    cw_ap = nc.dram_tensor("cw", [128, 160], f16, kind="ExternalInput").ap()
    out_ap = nc.dram_tensor("out", [M, NCP, 2], f32, kind="ExternalOutput").ap()

    Exp = mybir.ActivationFunctionType.Exp

    with tile.TileContext(nc) as tc:
        with (
            tc.tile_pool(name="consts", bufs=1) as consts,
            tc.tile_pool(name="d2p", bufs=2, space="PSUM") as d2_pool,
            tc.tile_pool(name="pop", bufs=2, space="PSUM") as po_pool,
        ):
            cw = consts.tile([128, 160], f16)
            rxsb = consts.tile([128, CH0_COLS], f16)
            rbf = consts.tile([128, 4 * 1536 + 256], f16)
            stage = consts.tile([128, M * NBLK * 2], f32)
            dum_i = consts.tile([128, 1], f32)
            dum_o = consts.tile([128, 1], f16)

            augw = cw[:, 0:128]
            wg = cw[:, 128:160]

            # memset first (dummy exp depends on it), then DMA triggers on
            # the vector queue; scalar queue = dummy + exps ONLY
            nc.vector.memset(dum_i[:], 0.0)
            nc.scalar.activation(dum_o[:], dum_i[:], Exp, scale=-1.0)
            # per-chain piece DMAs: a single batched transfer engages only ~2
            # DMA engines and serializes tiles 1-3 behind one completion sem;
            # per-piece DMAs spread across 10+ engines (scalar stays exp-only)
            nc.sync.dma_start(rxsb[0:10, 0:1024], rx_ap[0:10, 0:1024])
            nc.sync.dma_start(rxsb[64:74, 0:1024], rx_ap[64:74, 0:1024])
            nc.sync.dma_start(cw[:], cw_ap[:])
            nc.sync.dma_start(rxsb[0:10, 1024:2048], rx_ap[0:10, 1024:2048])
            nc.sync.dma_start(rxsb[64:74, 1024:2048], rx_ap[64:74, 1024:2048])
            nc.sync.dma_start(rxsb[0:10, 2048:CH0_COLS], rx_ap[0:10, 2048:CH0_COLS])
            nc.sync.dma_start(rxsb[64:74, 2048:4096], rx_ap[64:74, 2048:4096])
            nc.gpsimd.dma_start(rxsb[32:42, 0:1024], rx_ap[32:42, 0:1024])
            nc.gpsimd.dma_start(rxsb[32:42, 1024:2048], rx_ap[32:42, 1024:2048])
            nc.gpsimd.dma_start(rxsb[32:42, 2048:4096], rx_ap[32:42, 2048:4096])

            stv = stage[:].rearrange("p (m b d) -> p m b d", m=M, d=2)

            def mm1(t):
                # 3 chains x (A then B); A matmuls on distinct row-quadrants
                # run concurrently, B likewise (T2-verified: same-bank writes
                # from different quadrants are safe)
                d2 = d2_pool.tile([128, 1536], f32, tag="d2")
                for k, r0 in enumerate((0, 32, 64)):
                    nc.tensor.matmul(
                        d2[0:64, 512 * k : 512 * (k + 1)],
                        augw[r0 : r0 + 10, 0:64],
                        rxsb[r0 : r0 + 10, 1024 * t : 1024 * t + 512],
                        start=True,
                        stop=True,
                    )
                    nc.tensor.matmul(
                        d2[64:128, 512 * k : 512 * (k + 1)],
                        augw[r0 : r0 + 10, 64:128],
                        rxsb[r0 : r0 + 10, 1024 * t + 512 : 1024 * (t + 1)],
                        start=True,
                        stop=True,
                    )
                return d2

            def mm1_tail():
                d2 = d2_pool.tile([128, 1536], f32, tag="d2")
                nc.tensor.matmul(
                    d2[0:64, 0:128],
                    augw[0:10, 0:64],
                    rxsb[0:10, 4096:4224],
                    start=True,
                    stop=True,
                )
                nc.tensor.matmul(
                    d2[64:128, 0:128],
                    augw[0:10, 64:128],
                    rxsb[0:10, 4224:4352],
                    start=True,
                    stop=True,
                )
                return d2

            def mm2(t):
                # one matmul per col-block computes both packed blocks
                po = po_pool.tile([128, 32 * 12], f32, tag="po")
                for cb in range(12):
                    nc.tensor.matmul(
                        po[:, 32 * cb : 32 * cb + 32],
                        rbf[:, 1536 * t + 128 * cb : 1536 * t + 128 * cb + 128],
                        wg[:, 0:32],
                        start=True,
                        stop=True,
                    )
                pov = po[:].rearrange("p (i a m d) -> p m (i a) d", a=2, m=8, d=2)
                nc.vector.tensor_copy(
                    stv[:, :, 24 * t : 24 * t + 24, :], pov[:, 0:M, 0:24, :]
                )

            def mm2_tail():
                po = po_pool.tile([128, 32 * 12], f32, tag="po")
                nc.tensor.matmul(
                    po[:, 0:32], rbf[:, 6144:6272], wg[:, 0:32],
                    start=True, stop=True,
                )
                pov = po[:].rearrange("p (i a m d) -> p m (i a) d", a=2, m=8, d=2)
                nc.vector.tensor_copy(stv[:, :, 96:98, :], pov[:, 0:M, 0:2, :])

            def wave(blo, bhi):
                dst = out_ap.rearrange("m (p b) d -> p m b d", p=128)[
                    :, :, blo:bhi, :
                ]
                nc.sync.dma_start(dst, stv[:, :, blo:bhi, :])

            def do_exp(t, d2):
                fd = 1536 if t < NT else 128
                off = 1536 * t
                nc.scalar.activation(
                    rbf[:, off : off + fd], d2[:, 0:fd], Exp, scale=-1.0
                )

            d2 = mm1(0)
            do_exp(0, d2)
            d2 = mm1(1)
            do_exp(1, d2)
            for t in range(2, NT + 1):
                d2 = mm1(t) if t < NT else mm1_tail()
                do_exp(t, d2)
                mm2(t - 2)
                if t == 3:
                    wave(0, 48)
            mm2(NT - 1)
            mm2_tail()
            wave(48, NBLK)

    nc.compile()
    return nc


def _fit_anchors(centers, weights, xmax):
    """Interpolative decomposition of K(x, c) over a dense grid + LS fit of
    the 10 target functions on the selected anchor columns."""
    L = max(5.1, xmax + 0.35)
    ng = 96
    g1 = np.linspace(-L, L, ng)
    G2 = np.stack(np.meshgrid(g1, g1, indexing="ij"), -1).reshape(-1, 2)
    Kg = np.exp(-((G2[:, None, :] - centers[None, :, :]) ** 2).sum(-1))

    res = Kg.copy()
    sel = []
    for _ in range(R):
        j = int(np.argmax((res * res).sum(0)))
        sel.append(j)
        q = res[:, j].copy()
        nq = float(np.linalg.norm(q))
        if nq < 1e-12:
            break
        q /= nq
        res -= np.outer(q, q @ res)
    while len(sel) < R:          # degenerate guard: pad with repeats
        sel.append(sel[-1])

    V = weights.transpose(1, 0, 2).reshape(C, 10).astype(np.float64)
    F = SIGMA2 * (Kg @ V)
    A = Kg[:, sel]
    GA = A.T @ A + 1e-12 * np.eye(R)
    Gc = np.linalg.solve(GA, A.T @ F)          # [R, 10]
    return centers[sel].astype(np.float32), Gc.astype(np.float32)


def _host_prep(x, centers, weights):
    x = np.ascontiguousarray(np.asarray(x, dtype=np.float32))
    centers = np.asarray(centers, dtype=np.float32)
    weights = np.asarray(weights, dtype=np.float32)

    anchors, Gc = _fit_anchors(centers, weights, float(np.abs(x).max()))

    xp = np.zeros((NCORES * NCP, 2), np.float32)
    xp[:N] = x

    # x-side features, fp16 hi/lo split
    xh = xp.astype(np.float16)
    xl = (xp - xh.astype(np.float32)).astype(np.float16)
    x2 = np.sum(xp * xp, axis=1, dtype=np.float32)
    x2h = x2.astype(np.float16)
    x2l = (x2 - x2h.astype(np.float32)).astype(np.float16)
    ones = np.ones(NCORES * NCP, np.float16)
    feats = np.stack([
        xh[:, 0], xh[:, 0], xl[:, 0], xh[:, 1], xh[:, 1], xl[:, 1],
        x2h, x2l, ones, ones,
    ])  # [10, NCORES*NCP]
    fv = feats.reshape(10, NCORES, 128, NBLK)  # [k, core, j(p), b]

    # chain k, tile t: A-blocks {24t+2cb} then B-blocks {24t+2cb+1} for
    # cb in [4k, 4k+4); chain0 tail: blocks 96 (A), 97 (B)
    chains = []
    for k in range(3):
        blks = []
        for t in range(NT):
            blks += [24 * t + 2 * cb for cb in range(4 * k, 4 * k + 4)]
            blks += [24 * t + 2 * cb + 1 for cb in range(4 * k, 4 * k + 4)]
        if k == 0:
            blks += [96, 97]
        rc = fv[:, :, :, blks]                       # [10, core, j, nb]
        rc = np.ascontiguousarray(rc.transpose(1, 0, 3, 2))
        chains.append(rc.reshape(NCORES, 10, -1))    # [core, 10, 128*nb]
    rxp = np.zeros((NCORES, 74, CH0_COLS), np.float16)
    rxp[:, 0:10, :] = chains[0]
    rxp[:, 32:42, 0:CH_COLS] = chains[1]
    rxp[:, 64:74, 0:CH_COLS] = chains[2]

    # anchor-side features (both column halves hold the same 64 anchors)
    ch = anchors.astype(np.float16)
    cl = (anchors - ch.astype(np.float32)).astype(np.float16)
    c2 = np.sum(anchors * anchors, axis=1, dtype=np.float32)
    c2h = c2.astype(np.float16)
    c2l = (c2 - c2h.astype(np.float32)).astype(np.float16)
    onesR = np.ones(R, np.float16)
    cf = np.stack([
        -2 * ch[:, 0], -2 * cl[:, 0], -2 * ch[:, 0],
        -2 * ch[:, 1], -2 * cl[:, 1], -2 * ch[:, 1],
        onesR, onesR, c2h, c2l,
    ])  # [10, 64]
    cwm = np.zeros((128, 160), np.float16)
    for r0 in (0, 32, 64):
        cwm[r0 : r0 + 10, 0:64] = cf
        cwm[r0 : r0 + 10, 64:128] = cf
    # wg cols 0:16 = [G; 0] (A blocks), 16:32 = [0; G] (B blocks): one K=128
    # matmul per col-block computes both packed point-sets; the zero half
    # cancels the other set's rows.
    cwm[0:R, 128:138] = Gc.astype(np.float16)
    cwm[64 : 64 + R, 144:154] = Gc.astype(np.float16)
    return rxp, cwm


def kernel(x, centers, weights):
    if "nc" not in _CACHE:
        _CACHE["nc"] = _build()
    nc = _CACHE["nc"]
    rxp, cwm = _host_prep(x, centers, weights)
    in_maps = [{"rx": rxp[i], "cw": cwm} for i in range(NCORES)]
    res = run_bass_kernel_spmd(nc, in_maps, list(range(NCORES)))
    outs = np.concatenate([res.results[i]["out"] for i in range(NCORES)], axis=1)
    return np.ascontiguousarray(outs[:, :N, :])


# revision 12
# speedup vs baseline: 1.2521x; 1.0071x over previous
"""EnsembleRBF Trainium2 kernel: out[m,n,d] = sum_c exp(-||x_n - c_c||^2) * sigma^2 * w[m,c,d].

Rank-reduced design:
  The output is 10 fixed smooth functions f_{m,d}(x) = sigma^2 * sum_c
  exp(-||x-c_c||^2) w[m,c,d]. On host, select R=64 of the 256 kernel columns
  by greedy pivoted-QR over a dense x-grid (interpolative decomposition) and
  least-squares fit coefficients G[64,10] so sum_r exp(-||x-a_r||^2) G[r,:]
  matches all 10 targets to ~2e-4 (gate is 2e-2). Device exp work shrinks
  4x: [n, 64] instead of [n, 256].

  Data-parallel along N across 8 cores, NCP = 12544 = 128 x 98 blocks,
  n = p*98 + b. Two point-sets packed on the partition axis keep ACT full
  width: d2 tile [128, 1536] fp32 PSUM (3 banks), partitions 0:64 = 64
  anchors x even blocks (set A), 64:128 = anchors x odd blocks (set B);
  col-block cb of tile t holds blocks 24t+2cb (A) / 24t+2cb+1 (B).

  MM1 (PE): 6 matmuls per tile (FD=512, K=10 fp16 hi/lo features); three rx
  chains at partition bases 0/32/64 rotate across row-quadrants so the three
  A matmuls run concurrently (per-subarray row tiling), B likewise.

  ACT: one exp(-d2) per tile (FD=1536) = the stream bottleneck; 4 full tiles
  + 1 tail (2 blocks, FD=128). The scalar queue carries ONLY the dummy exp
  (hoists the ~2.7us table load to t~0) and the exps - DMA triggers cost
  ~740ns of sequencer time each and must live on other queues.

  MM2 (PE): ONE matmul per col-block computes BOTH packed blocks: stationary
  rbf[:, 128cb:+128] (K=128), moving wg[:, 0:32] with cols 0:16 = [G; 0]
  (kills the B rows) and 16:32 = [0; G] (kills A) -> po[j, 32cb+16a+2m+d].
  49 matmuls total instead of 98; per-PE-instruction cost (~120ns dispatch+
  LDW+MM) dominates, so count is everything. (K=64 stationaries at partition
  base 64 crash the HW - all-row zero-padded contraction is the workaround.)

  DVE: po -> stage (m,b,d)-major fp32. 5 output waves on the gpsimd queue,
  each one 4D-AP DMA covering all 5 models ([p][m][b][d] element order).
"""
import numpy as np

import concourse.bass as bass
import concourse.tile as tile
from concourse import bacc, mybir
from concourse.bass_utils import run_bass_kernel_spmd

N, C, D, M = 100000, 256, 2, 5
SIGMA2 = 0.0625
NCORES = 8
NCP = 12544          # padded rows per core (128 x 98)
NBLK = NCP // 128    # 98 blocks, n = p*98 + b
R = 64               # anchor count
f32 = mybir.dt.float32
f16 = mybir.dt.float16

NT = 4               # full tiles (24 blocks each); tail = blocks 96,97
CH0_COLS = 4 * 1024 + 256   # chain0 also carries the tail blocks
CH_COLS = 4 * 1024

_CACHE = {}


def _build():
    nc = bacc.Bacc("TRN2", target_bir_lowering=False, debug=False, num_devices=NCORES)
    rx_ap = nc.dram_tensor("rx", [74, CH0_COLS], f16, kind="ExternalInput").ap()
    cw_ap = nc.dram_tensor("cw", [128, 160], f16, kind="ExternalInput").ap()
    # [p][b][m][d] layout: output waves become 128 contiguous ~2KB runs
    # instead of 640 x 192B (descriptor count dominates the 8-core flush);
    # the host gather untransposes.
    out_ap = nc.dram_tensor("out", [128, NBLK, M, 2], f32, kind="ExternalOutput").ap()

    Exp = mybir.ActivationFunctionType.Exp

    with tile.TileContext(nc) as tc:
        with (
            tc.tile_pool(name="consts", bufs=1) as consts,
            tc.tile_pool(name="d2p", bufs=2, space="PSUM") as d2_pool,
            tc.tile_pool(name="pop", bufs=2, space="PSUM") as po_pool,
        ):
            cw = consts.tile([128, 160], f16)
            rxsb = consts.tile([128, CH0_COLS], f16)
            rbf = consts.tile([128, 4 * 1536 + 256], f16)
            stage = consts.tile([128, M * NBLK * 2], f32)
            dum_i = consts.tile([128, 1], f32)
            dum_o = consts.tile([128, 1], f16)

            augw = cw[:, 0:128]
            wg = cw[:, 128:160]

            # memset first (dummy exp depends on it), then DMA triggers on
            # the vector queue; scalar queue = dummy + exps ONLY
            nc.vector.memset(dum_i[:], 0.0)
            nc.scalar.activation(dum_o[:], dum_i[:], Exp, scale=-1.0)
            # per-chain piece DMAs: a single batched transfer engages only ~2
            # DMA engines and serializes tiles 1-3 behind one completion sem;
            # per-piece DMAs spread across 10+ engines (scalar stays exp-only)
            nc.sync.dma_start(rxsb[0:10, 0:1024], rx_ap[0:10, 0:1024])
            nc.sync.dma_start(rxsb[64:74, 0:1024], rx_ap[64:74, 0:1024])
            nc.sync.dma_start(cw[:], cw_ap[:])
            nc.sync.dma_start(rxsb[0:10, 1024:2048], rx_ap[0:10, 1024:2048])
            nc.sync.dma_start(rxsb[64:74, 1024:2048], rx_ap[64:74, 1024:2048])
            nc.sync.dma_start(rxsb[0:10, 2048:CH0_COLS], rx_ap[0:10, 2048:CH0_COLS])
            nc.sync.dma_start(rxsb[64:74, 2048:4096], rx_ap[64:74, 2048:4096])
            nc.gpsimd.dma_start(rxsb[32:42, 0:1024], rx_ap[32:42, 0:1024])
            nc.gpsimd.dma_start(rxsb[32:42, 1024:2048], rx_ap[32:42, 1024:2048])
            nc.gpsimd.dma_start(rxsb[32:42, 2048:4096], rx_ap[32:42, 2048:4096])

            stv = stage[:].rearrange("p (b m d) -> p b m d", m=M, d=2)

            def mm1(t):
                # 3 chains x (A then B); A matmuls on distinct row-quadrants
                # run concurrently, B likewise (T2-verified: same-bank writes
                # from different quadrants are safe)
                d2 = d2_pool.tile([128, 1536], f32, tag="d2")
                for k, r0 in enumerate((0, 32, 64)):
                    nc.tensor.matmul(
                        d2[0:64, 512 * k : 512 * (k + 1)],
                        augw[r0 : r0 + 10, 0:64],
                        rxsb[r0 : r0 + 10, 1024 * t : 1024 * t + 512],
                        start=True,
                        stop=True,
                    )
                    nc.tensor.matmul(
                        d2[64:128, 512 * k : 512 * (k + 1)],
                        augw[r0 : r0 + 10, 64:128],
                        rxsb[r0 : r0 + 10, 1024 * t + 512 : 1024 * (t + 1)],
                        start=True,
                        stop=True,
                    )
                return d2

            def mm1_tail():
                d2 = d2_pool.tile([128, 1536], f32, tag="d2")
                nc.tensor.matmul(
                    d2[0:64, 0:128],
                    augw[0:10, 0:64],
                    rxsb[0:10, 4096:4224],
                    start=True,
                    stop=True,
                )
                nc.tensor.matmul(
                    d2[64:128, 0:128],
                    augw[0:10, 64:128],
                    rxsb[0:10, 4224:4352],
                    start=True,
                    stop=True,
                )
                return d2

            def mm2(t):
                # one matmul per col-block computes both packed blocks
                po = po_pool.tile([128, 32 * 12], f32, tag="po")
                for cb in range(12):
                    nc.tensor.matmul(
                        po[:, 32 * cb : 32 * cb + 32],
                        rbf[:, 1536 * t + 128 * cb : 1536 * t + 128 * cb + 128],
                        wg[:, 0:32],
                        start=True,
                        stop=True,
                    )
                pov = po[:].rearrange("p (i a m d) -> p (i a) m d", a=2, m=8, d=2)
                nc.vector.tensor_copy(
                    stv[:, 24 * t : 24 * t + 24, :, :], pov[:, 0:24, 0:M, :]
                )

            def mm2_tail():
                po = po_pool.tile([128, 32 * 12], f32, tag="po")
                nc.tensor.matmul(
                    po[:, 0:32], rbf[:, 6144:6272], wg[:, 0:32],
                    start=True, stop=True,
                )
                pov = po[:].rearrange("p (i a m d) -> p (i a) m d", a=2, m=8, d=2)
                nc.vector.tensor_copy(stv[:, 96:98, :, :], pov[:, 0:2, 0:M, :])

            def wave(blo, bhi):
                nc.sync.dma_start(
                    out_ap[:, blo:bhi, :, :], stv[:, blo:bhi, :, :]
                )

            def do_exp(t, d2):
                fd = 1536 if t < NT else 128
                off = 1536 * t
                nc.scalar.activation(
                    rbf[:, off : off + fd], d2[:, 0:fd], Exp, scale=-1.0
                )

            d2 = mm1(0)
            do_exp(0, d2)
            d2 = mm1(1)
            do_exp(1, d2)
            for t in range(2, NT + 1):
                d2 = mm1(t) if t < NT else mm1_tail()
                do_exp(t, d2)
                mm2(t - 2)
                if t == 3:
                    wave(0, 48)
            mm2(NT - 1)
            mm2_tail()
            wave(48, NBLK)

    nc.compile()
    return nc


def _fit_anchors(centers, weights, xmax):
    """Interpolative decomposition of K(x, c) over a dense grid + LS fit of
    the 10 target functions on the selected anchor columns."""
    L = max(5.1, xmax + 0.35)
    ng = 96
    g1 = np.linspace(-L, L, ng)
    G2 = np.stack(np.meshgrid(g1, g1, indexing="ij"), -1).reshape(-1, 2)
    Kg = np.exp(-((G2[:, None, :] - centers[None, :, :]) ** 2).sum(-1))

    res = Kg.copy()
    sel = []
    for _ in range(R):
        j = int(np.argmax((res * res).sum(0)))
        sel.append(j)
        q = res[:, j].copy()
        nq = float(np.linalg.norm(q))
        if nq < 1e-12:
            break
        q /= nq
        res -= np.outer(q, q @ res)
    while len(sel) < R:          # degenerate guard: pad with repeats
        sel.append(sel[-1])

    V = weights.transpose(1, 0, 2).reshape(C, 10).astype(np.float64)
    F = SIGMA2 * (Kg @ V)
    A = Kg[:, sel]
    GA = A.T @ A + 1e-12 * np.eye(R)
    Gc = np.linalg.solve(GA, A.T @ F)          # [R, 10]
    return centers[sel].astype(np.float32), Gc.astype(np.float32)


def _host_prep(x, centers, weights):
    x = np.ascontiguousarray(np.asarray(x, dtype=np.float32))
    centers = np.asarray(centers, dtype=np.float32)
    weights = np.asarray(weights, dtype=np.float32)

    anchors, Gc = _fit_anchors(centers, weights, float(np.abs(x).max()))

    xp = np.zeros((NCORES * NCP, 2), np.float32)
    xp[:N] = x

    # x-side features, fp16 hi/lo split
    xh = xp.astype(np.float16)
    xl = (xp - xh.astype(np.float32)).astype(np.float16)
    x2 = np.sum(xp * xp, axis=1, dtype=np.float32)
    x2h = x2.astype(np.float16)
    x2l = (x2 - x2h.astype(np.float32)).astype(np.float16)
    ones = np.ones(NCORES * NCP, np.float16)
    feats = np.stack([
        xh[:, 0], xh[:, 0], xl[:, 0], xh[:, 1], xh[:, 1], xl[:, 1],
        x2h, x2l, ones, ones,
    ])  # [10, NCORES*NCP]
    fv = feats.reshape(10, NCORES, 128, NBLK)  # [k, core, j(p), b]

    # chain k, tile t: A-blocks {24t+2cb} then B-blocks {24t+2cb+1} for
    # cb in [4k, 4k+4); chain0 tail: blocks 96 (A), 97 (B)
    chains = []
    for k in range(3):
        blks = []
        for t in range(NT):
            blks += [24 * t + 2 * cb for cb in range(4 * k, 4 * k + 4)]
            blks += [24 * t + 2 * cb + 1 for cb in range(4 * k, 4 * k + 4)]
        if k == 0:
            blks += [96, 97]
        rc = fv[:, :, :, blks]                       # [10, core, j, nb]
        rc = np.ascontiguousarray(rc.transpose(1, 0, 3, 2))
        chains.append(rc.reshape(NCORES, 10, -1))    # [core, 10, 128*nb]
    rxp = np.zeros((NCORES, 74, CH0_COLS), np.float16)
    rxp[:, 0:10, :] = chains[0]
    rxp[:, 32:42, 0:CH_COLS] = chains[1]
    rxp[:, 64:74, 0:CH_COLS] = chains[2]

    # anchor-side features (both column halves hold the same 64 anchors)
    ch = anchors.astype(np.float16)
    cl = (anchors - ch.astype(np.float32)).astype(np.float16)
    c2 = np.sum(anchors * anchors, axis=1, dtype=np.float32)
    c2h = c2.astype(np.float16)
    c2l = (c2 - c2h.astype(np.float32)).astype(np.float16)
    onesR = np.ones(R, np.float16)
    cf = np.stack([
        -2 * ch[:, 0], -2 * cl[:, 0], -2 * ch[:, 0],
        -2 * ch[:, 1], -2 * cl[:, 1], -2 * ch[:, 1],
        onesR, onesR, c2h, c2l,
    ])  # [10, 64]
    cwm = np.zeros((128, 160), np.float16)
    for r0 in (0, 32, 64):
        cwm[r0 : r0 + 10, 0:64] = cf
        cwm[r0 : r0 + 10, 64:128] = cf
    # wg cols 0:16 = [G; 0] (A blocks), 16:32 = [0; G] (B blocks): one K=128
    # matmul per col-block computes both packed point-sets; the zero half
    # cancels the other set's rows.
    cwm[0:R, 128:138] = Gc.astype(np.float16)
    cwm[64 : 64 + R, 144:154] = Gc.astype(np.float16)
    return rxp, cwm


def kernel(x, centers, weights):
    if "nc" not in _CACHE:
        _CACHE["nc"] = _build()
    nc = _CACHE["nc"]
    rxp, cwm = _host_prep(x, centers, weights)
    in_maps = [{"rx": rxp[i], "cw": cwm} for i in range(NCORES)]
    res = run_bass_kernel_spmd(nc, in_maps, list(range(NCORES)))
    # per-core out is [p, b, m, d]; n = p*98 + b
    outs = np.concatenate(
        [
            res.results[i]["out"].transpose(2, 0, 1, 3).reshape(M, NCP, 2)
            for i in range(NCORES)
        ],
        axis=1,
    )
    return np.ascontiguousarray(outs[:, :N, :])
